# revision 14
# baseline (speedup 1.0000x reference)
"""Trainium2 Bass kernel for nn_GATPredictor (2-layer multi-head GAT + MLP).

kernel(**inputs) takes FULL unsharded numpy inputs, returns the FULL
(50000,) float32 output.  Internally: 8-way dst-node sharding (degree
round-robin), replicated weights, one AllGather of layer-1 src features,
padded per-(group,half) neighbor-slot gathers via dma_gather (int16 idx,
two half-tables), two-pass per-group softmax with the normalization
folded into the edge weights, per-edge weighting split 3 heads on DVE +
1 head on ACT, and PSUM-accumulated identity matmuls (bf16) for the
weighted scatter-add.
"""

import numpy as np

N = 50000
E = 800000
F_IN = 64
H = 4
C = 128
NEG = 0.2
R = 8
PER = N // R
DUM = 22
CHUNK = PER + DUM  # 6272
G = CHUNK // 128  # 49
NPOS = R * CHUNK  # 50176
HALF = NPOS // 2  # 25088
CJ = 12
NB = NPOS // 128  # 392
WIN = 32768
BASES = [0, (NPOS - WIN) // 2, NPOS - WIN]  # [0, 8704, 17408]
NW = 3
# per-window pad index: first dummy (rank-chunk start) row inside the window
PADIDX = [min(x for x in range(0, NPOS, CHUNK) if x >= b) - b for b in BASES]

ROWF0 = 256  # table0 row (bf16): [h(128) | asrc(4) | pad] -> 512B
ROWF1 = 640  # table1 row (bf16): [x1(512) | asrc(4) | pad] -> 1280B


# ---------------------------------------------------------------- host prep
def _prep_graph(edge_index, edge_weight):
    src = np.concatenate([np.asarray(edge_index[0], np.int64), np.arange(N)])
    dst = np.concatenate([np.asarray(edge_index[1], np.int64), np.arange(N)])
    ew = np.concatenate([np.asarray(edge_weight, np.float32), np.ones(N, np.float32)])

    deg = np.bincount(dst, minlength=N)
    gorder = np.argsort(deg, kind="stable")
    rank_of_node = np.empty(N, np.int64)
    rank_of_node[gorder] = np.arange(N) % R

    pos_of_node = np.empty(N, np.int64)
    perm_per_rank = []
    for r in range(R):
        nodes = np.where(rank_of_node == r)[0]
        order = np.argsort(deg[nodes], kind="stable")
        sorted_nodes = nodes[order]
        perm_per_rank.append(sorted_nodes)
        pos_of_node[sorted_nodes] = CHUNK * r + DUM + np.arange(nodes.shape[0])

    src_pos = pos_of_node[src]
    dst_pos = pos_of_node[dst]

    basesv = np.asarray(BASES)
    lo = np.searchsorted(basesv + WIN, src_pos, side="right")
    hi = np.searchsorted(basesv, src_pos, side="right") - 1
    assert (lo <= hi).all() and (lo >= 0).all() and (hi < NW).all()

    # per-lane rank in pos order within each dst
    order = np.lexsort((src_pos, dst_pos))
    dpo, loo, hio, spo = dst_pos[order], lo[order], hi[order], src_pos[order]
    swo = ew[order]
    tot = np.bincount(dst_pos, minlength=NPOS)
    change = np.ones(dpo.shape[0], np.bool_)
    change[1:] = dpo[1:] != dpo[:-1]
    run_start = np.maximum.accumulate(np.where(change, np.arange(dpo.shape[0]), 0))
    k_in = np.arange(dpo.shape[0]) - run_start

    # Hall capacities per group
    lanes = lambda cnt: cnt.reshape(R, G, 128)
    H0 = lanes(np.bincount(dpo[hio <= 0], minlength=NPOS)).max(axis=(0, 2))
    H1 = lanes(np.bincount(dpo[hio <= 1], minlength=NPOS)).max(axis=(0, 2))
    L2 = lanes(np.bincount(dpo[loo >= 2], minlength=NPOS)).max(axis=(0, 2))
    L1 = lanes(np.bincount(dpo[loo >= 1], minlength=NPOS)).max(axis=(0, 2))
    M1 = lanes(np.bincount(dpo[(loo == 1) & (hio == 1)], minlength=NPOS)).max(axis=(0, 2))
    Tm = lanes(tot).max(axis=(0, 2))
    C0 = H0
    C2 = L2
    C1 = np.maximum.reduce([H1 - C0, L1 - C2, Tm - C0 - C2, M1, np.zeros(G, np.int64)])

    # earliest-fit per lane: window w takes (in pos order) up to C_w of the
    # not-yet-assigned edges with lo <= w
    g_of = (dpo % CHUNK) // 128

    def masked_rank(mask):
        # rank of each True element within its (dst) run, counting only True
        cm = np.cumsum(mask)
        base = np.zeros_like(cm)
        base[run_start] = cm[run_start] - mask[run_start]
        base = np.maximum.accumulate(np.where(change, base, 0))
        return cm - 1 - base

    wsel = np.full(dpo.shape[0], NW - 1, np.int64)
    unassigned = np.ones(dpo.shape[0], np.bool_)
    caps = [C0, C1]
    for w in range(NW - 1):
        elig = unassigned & (loo <= w)
        rk = masked_rank(elig)
        take = elig & (rk < caps[w][g_of])
        wsel[take] = w
        unassigned &= ~take
    assert (wsel >= loo).all() and (wsel <= hio).all(), "window assignment infeasible" 

    # actual per-(lane,window) loads and slot within (dst, window)
    loads = np.zeros((NPOS, NW), np.int64)
    np.add.at(loads, (dpo, wsel), 1)
    Duni = loads.reshape(R, G, 128, NW).max(axis=(0, 2))  # (G, NW)

    call_plan = []
    for g in range(G):
        for w in range(NW):
            left = int(Duni[g, w])
            while left > 0:
                c = min(CJ, left)
                call_plan.append((g, w, c))
                left -= c
    tot_cols = int(Duni.sum())

    colbase = np.zeros((G, NW), np.int64)
    acc = 0
    for g in range(G):
        for w in range(NW):
            colbase[g, w] = acc
            acc += int(Duni[g, w])
    assert acc == tot_cols

    # slot index within (dst, window): order edges by (dst, window, pos)
    order2 = np.lexsort((spo, wsel, dpo))
    sd, sw_, si_pos, swt = dpo[order2], wsel[order2], spo[order2], swo[order2]
    si = si_pos - basesv[sw_]
    assert (si >= 0).all() and (si < WIN).all()
    key = sd * NW + sw_
    change = np.ones(key.shape[0], np.bool_)
    change[1:] = key[1:] != key[:-1]
    run_start = np.maximum.accumulate(np.where(change, np.arange(key.shape[0]), 0))
    slot_k = np.arange(key.shape[0]) - run_start

    e_rank = sd // CHUNK
    e_loc = sd - e_rank * CHUNK
    e_g = e_loc // 128
    e_p = e_loc % 128
    e_col = colbase[e_g, sw_] + slot_k

    gidx = np.zeros((R, 128, tot_cols), np.int32)
    ewt = np.zeros((R, 128, tot_cols), np.float32)
    for g in range(G):
        for w in range(NW):
            gidx[:, :, colbase[g, w] : colbase[g, w] + Duni[g, w]] = PADIDX[w]
    gidx[e_rank, e_p, e_col] = si
    ewt[e_rank, e_p, e_col] = swt

    idx16 = np.zeros((R, 128, 8 * tot_cols), np.int16)
    cursor = {(g, w): 0 for g in range(G) for w in range(NW)}
    off = 0
    for g, w, c in call_plan:
        cb = colbase[g, w] + cursor[(g, w)]
        cursor[(g, w)] += c
        cols = gidx[:, :, cb : cb + c]  # (R, 128, c)
        stream = cols.transpose(0, 2, 1).reshape(R, -1)
        wv = stream.reshape(R, c * 8, 16).transpose(0, 2, 1).astype(np.int16)
        idx16[:, :, off : off + 8 * c] = np.tile(wv, (1, 8, 1))
        off += 8 * c
    assert off == 8 * tot_cols

    meta = dict(call_plan=call_plan, tot_cols=tot_cols)
    return perm_per_rank, gidx, ewt, idx16, meta


def _prep_weights(W_in, b_in, lin0, a_src0, a_dst0, lin1, a_src1, a_dst1):
    def fold(lin, a):
        return np.einsum(
            "ihc,hc->ih",
            lin.reshape(lin.shape[0], H, C).astype(np.float64),
            a.astype(np.float64),
        ).astype(np.float32)

    w_src0, w_dst0 = fold(lin0, a_src0), fold(lin0, a_dst0)
    rhs0 = np.zeros((128, 136), np.float32)
    rhs0[:F_IN, :C] = W_in
    rhs0[:F_IN, C : C + 4] = W_in @ w_src0
    rhs0[:F_IN, C + 4 :] = W_in @ w_dst0
    rhs0[F_IN, :C] = b_in
    rhs0[F_IN, C : C + 4] = b_in @ w_src0
    rhs0[F_IN, C + 4 :] = b_in @ w_dst0
    rhs0L = np.zeros((128, 132), np.float32)
    rhs0L[:F_IN, :C] = W_in
    rhs0L[:F_IN, C:] = W_in @ w_dst0
    rhs0L[F_IN, :C] = b_in
    rhs0L[F_IN, C:] = b_in @ w_dst0
    wa1 = np.concatenate([fold(lin1, a_src1), fold(lin1, a_dst1)], axis=1)
    return rhs0, rhs0L, wa1.astype(np.float32)


def _build_xpt_blocks(X, perm_per_rank):
    xpt = np.zeros((128, NPOS), np.float32)
    xpt[F_IN, :] = 1.0
    for r in range(R):
        cols = CHUNK * r + DUM + np.arange(perm_per_rank[r].shape[0])
        xpt[:F_IN, cols] = X[perm_per_rank[r]].T
    return xpt.reshape(128, NB, 128).transpose(1, 0, 2).copy()


# ---------------------------------------------------------------- bass build
def _build_nc(call_plan, tot_cols):
    import concourse.bacc as bacc
    import concourse.bass_isa as bass_isa
    import concourse.mybir as mybir
    import concourse.tile as tile
    from concourse.masks import make_identity

    f32 = mybir.dt.float32
    bf16 = mybir.dt.bfloat16
    i16 = mybir.dt.int16
    AF = mybir.ActivationFunctionType
    OP = mybir.AluOpType
    AX = mybir.AxisListType

    nc = bacc.Bacc(
        "TRN2",
        target_bir_lowering=False,
        debug=False,
        enable_asserts=False,
        num_devices=R,
        num_swdge_queues=4,
    )

    xptb = nc.dram_tensor("xptb", [NB, 128, 128], bf16, kind="ExternalInput")
    xptl = nc.dram_tensor("xptl", [G, 128, 128], bf16, kind="ExternalInput")
    rhs0_d = nc.dram_tensor("rhs0", [128, 136], bf16, kind="ExternalInput")
    rhs0l_d = nc.dram_tensor("rhs0l", [128, 132], bf16, kind="ExternalInput")
    lin0_d = nc.dram_tensor("lin0", [128, 512], bf16, kind="ExternalInput")
    lin1_d = nc.dram_tensor("lin1", [512, 512], bf16, kind="ExternalInput")
    wa1_d = nc.dram_tensor("wa1", [512, 8], bf16, kind="ExternalInput")
    wp1_d = nc.dram_tensor("wp1", [640, 128], bf16, kind="ExternalInput")
    brow_d = nc.dram_tensor("brow", [1, 1280], f32, kind="ExternalInput")
    bp2_d = nc.dram_tensor("bp2", [1, 1], f32, kind="ExternalInput")
    idx_d = nc.dram_tensor("idx16", [128, 8 * tot_cols], i16, kind="ExternalInput")
    ew_d = nc.dram_tensor("ewt", [128, tot_cols], f32, kind="ExternalInput")
    y_d = nc.dram_tensor("y", [128, G], f32, kind="ExternalOutput")

    # group -> list of (idx_off8, col, ncols, half); order mirrors host prep
    plan_by_group = [[] for _ in range(G)]
    colbase = {}
    acc = 0
    for g in range(G):
        for h in range(NW):
            cols_gh = sum(c for (gg, hh, c) in call_plan if gg == g and hh == h)
            colbase[(g, h)] = acc
            acc += cols_gh
    cursor = {(g, h): 0 for g in range(G) for h in range(NW)}
    off8 = 0
    for g, h, c in call_plan:
        col = colbase[(g, h)] + cursor[(g, h)]
        cursor[(g, h)] += c
        plan_by_group[g].append((off8, col, c, h))
        off8 += 8 * c

    with tile.TileContext(nc) as tc:
        with tc.tile_pool(name="dram", bufs=1, space="DRAM") as dram, tc.tile_pool(
            name="const", bufs=1
        ) as cp:
            table0 = dram.tile([NPOS, ROWF0], bf16)
            h_res_d = dram.tile([G, 128, 128], bf16)
            agin = dram.tile([CHUNK, ROWF1], bf16)
            table1 = dram.tile([NPOS, ROWF1], bf16)

            ident = cp.tile([128, 128], f32)
            make_identity(nc, ident[:])
            identb = cp.tile([128, 128], bf16)
            nc.vector.tensor_copy(out=identb[:], in_=ident[:])
            rhs0_s = cp.tile([128, 136], bf16)
            nc.sync.dma_start(out=rhs0_s[:], in_=rhs0_d[:, :])
            rhs0l_s = cp.tile([128, 132], bf16)
            nc.sync.dma_start(out=rhs0l_s[:], in_=rhs0l_d[:, :])
            lin0_s = cp.tile([128, 512], bf16)
            nc.sync.dma_start(out=lin0_s[:], in_=lin0_d[:, :])
            lin1_s = [cp.tile([128, 512], bf16, tag=f"lin1_{c}", name=f"lin1s{c}") for c in range(4)]
            wa1_s = [cp.tile([128, 8], bf16, tag=f"wa1_{c}", name=f"wa1s{c}") for c in range(4)]
            for c in range(4):
                nc.sync.dma_start(
                    out=lin1_s[c][:], in_=lin1_d[128 * c : 128 * (c + 1), :]
                )
                nc.sync.dma_start(
                    out=wa1_s[c][:], in_=wa1_d[128 * c : 128 * (c + 1), :]
                )
            wp1_s = [cp.tile([128, 128], bf16, tag=f"wp1_{c}", name=f"wp1s{c}") for c in range(5)]
            for c in range(5):
                nc.sync.dma_start(
                    out=wp1_s[c][:], in_=wp1_d[128 * c : 128 * (c + 1), :]
                )
            brow = cp.tile([128, 1280], f32)
            nc.sync.dma_start(out=brow[0:1, :], in_=brow_d[:, :])
            nc.gpsimd.partition_broadcast(brow[:], brow[0:1, :])
            bias0r = brow[:, 0:512]
            bias1r = brow[:, 512:1024]
            bp1r = brow[:, 1024:1152]
            wp2r = brow[:, 1152:1280]
            bp2t = cp.tile([128, 1], f32)
            nc.sync.dma_start(out=bp2t[0:1, :], in_=bp2_d[:, :])
            nc.gpsimd.partition_broadcast(bp2t[:], bp2t[0:1, :])
            idx_s = cp.tile([128, 8 * tot_cols], i16)
            nc.sync.dma_start(out=idx_s[:], in_=idx_d[:, :])
            ew_s = cp.tile([128, tot_cols], f32)
            nc.sync.dma_start(out=ew_s[:], in_=ew_d[:, :])
            adst0_s = cp.tile([128, G * 4], f32)
            adst1_s = cp.tile([128, G * 4], f32)
            ysb = cp.tile([128, G], f32)

            # ---------------- M0: replicated table0 + M0L local
            with nc.named_scope("m0"), tc.tile_pool(name="m0s", bufs=3) as mp, tc.tile_pool(
                name="m0p", bufs=2, space="PSUM"
            ) as mpp:
                for b in range(NB):
                    xb = mp.tile([128, 128], bf16, tag="xb")
                    nc.sync.dma_start(out=xb[:], in_=xptb[b, :, :])
                    ps = mpp.tile([128, 136], f32, tag="m0ps")
                    nc.tensor.matmul(
                        ps[:], lhsT=xb[:], rhs=rhs0_s[:], start=True, stop=True
                    )
                    stg = mp.tile([128, ROWF0], bf16, tag="stg0")
                    nc.scalar.copy(out=stg[:, 0:132], in_=ps[:, 0:132])
                    if b % G == 0:
                        nc.vector.memset(stg[0:1, 128:132], -1e30)
                    nc.sync.dma_start(
                        out=table0[128 * b : 128 * (b + 1), :], in_=stg[:]
                    )
                for g in range(G):
                    xb = mp.tile([128, 128], bf16, tag="xb")
                    nc.sync.dma_start(out=xb[:], in_=xptl[g, :, :])
                    ps = mpp.tile([128, 136], f32, tag="m0ps")
                    nc.tensor.matmul(
                        ps[:, 0:132], lhsT=xb[:], rhs=rhs0l_s[:], start=True, stop=True
                    )
                    stg = mp.tile([128, 128], bf16, tag="stgL")
                    nc.scalar.copy(out=stg[:], in_=ps[:, 0:128])
                    nc.sync.dma_start(out=h_res_d[g, :, :], in_=stg[:])
                    nc.vector.tensor_copy(
                        out=adst0_s[:, 4 * g : 4 * (g + 1)], in_=ps[:, 128:132]
                    )

            qn = [0]

            def gather(out_ap, half_ap, ioff, c, elem):
                nc.gpsimd.dma_gather(
                    out_ap=out_ap,
                    in_ap=half_ap,
                    idxs_ap=idx_s[:, ioff : ioff + 8 * c],
                    num_idxs=128 * c,
                    num_idxs_reg=128 * c,
                    elem_size=elem,
                    single_packet=False,
                    queue_num=qn[0] % 4,
                )
                qn[0] += 1

            # ---------------- L0 aggregation + M2 (fused per group)
            with nc.named_scope("l0"), tc.tile_pool(name="l0g", bufs=12) as gp0, tc.tile_pool(
                name="l0w", bufs=6
            ) as wp0, tc.tile_pool(name="l0e", bufs=12) as ep0, tc.tile_pool(
                name="l0t", bufs=4
            ) as tp0, tc.tile_pool(
                name="l0m", bufs=3
            ) as sp0, tc.tile_pool(
                name="l0acc", bufs=3, space="PSUM"
            ) as accp0, tc.tile_pool(
                name="l0out", bufs=1, space="PSUM"
            ) as outp0, tc.tile_pool(
                name="l0a", bufs=1, space="PSUM"
            ) as ap0, tc.tile_pool(
                name="l0tr", bufs=2, space="PSUM"
            ) as trp0:
                for g in range(G):
                    calls = plan_by_group[g]
                    ncols = sum(c for (_, _, c, _) in calls)
                    accps = accp0.tile([128, 512], f32, tag="acc")
                    s_acc = sp0.tile([128, 4], f32, tag="sacc")
                    nc.vector.memset(s_acc[:], 0.0)
                    work = []
                    # ---- pass 1: gather + alpha(exp) + s accumulation
                    for ioff, col, c, h in calls:
                        xg = gp0.tile([128, CJ, ROWF0], bf16, tag="xg")
                        gather(
                            xg[:, 0:c, :],
                            table0[BASES[h] : BASES[h] + WIN, :],
                            ioff,
                            c,
                            ROWF0,
                        )
                        et = ep0.tile([128, 4, CJ], f32, tag="et")
                        nc.vector.tensor_tensor(
                            out=et[:, :, 0:c],
                            in0=xg[:, 0:c, 128:132].rearrange("p c f -> p f c"),
                            in1=adst0_s[:, 4 * g : 4 * (g + 1)].to_broadcast(
                                [128, 4, c]
                            ),
                            op=OP.add,
                        )
                        lr = tp0.tile([128, 4, CJ], f32, tag="lr")
                        nc.vector.tensor_scalar(
                            out=lr[:, :, 0:c], in0=et[:, :, 0:c], scalar1=NEG,
                            scalar2=None, op0=OP.mult,
                        )
                        nc.vector.tensor_tensor(
                            out=et[:, :, 0:c], in0=et[:, :, 0:c], in1=lr[:, :, 0:c],
                            op=OP.max,
                        )
                        nc.scalar.activation(et[:, :, 0:c], et[:, :, 0:c], AF.Exp)
                        red = tp0.tile([128, 4], f32, tag="red")
                        nc.vector.tensor_reduce(
                            red[:], et[:, :, 0:c], axis=AX.X, op=OP.add
                        )
                        nc.vector.tensor_tensor(
                            out=s_acc[:], in0=s_acc[:], in1=red[:], op=OP.add
                        )
                        work.append((xg, et, col, c))
                    # ---- normalization folded into weights
                    srec = sp0.tile([128, 4], f32, tag="srec")
                    nc.vector.tensor_scalar(
                        out=srec[:], in0=s_acc[:], scalar1=1e-16, scalar2=None, op0=OP.add
                    )
                    nc.vector.reciprocal(srec[:], srec[:])
                    # ---- pass 2: wt = exp * ew * srec; weighted scatter-add
                    coli = 0
                    for xg, et, col, c in work:
                        nc.vector.tensor_tensor(
                            out=et[:, :, 0:c],
                            in0=et[:, :, 0:c],
                            in1=ew_s[:, None, col : col + c].to_broadcast([128, 4, c]),
                            op=OP.mult,
                        )
                        nc.vector.tensor_tensor(
                            out=et[:, :, 0:c],
                            in0=et[:, :, 0:c],
                            in1=srec[:, :, None].to_broadcast([128, 4, c]),
                            op=OP.mult,
                        )
                        for k in range(c):
                            wx = wp0.tile([128, 512], bf16, tag="wx")
                            nc.vector.tensor_tensor(
                                out=wx[:, 0:384].rearrange("p (h f) -> p h f", h=3),
                                in0=xg[:, k, 0:128][:, None, :].to_broadcast(
                                    [128, 3, 128]
                                ),
                                in1=et[:, 0:3, k].to_broadcast([128, 3, 128]),
                                op=OP.mult,
                            )
                            nc.scalar.activation(
                                wx[:, 384:512],
                                xg[:, k, 0:128],
                                AF.Copy,
                                scale=et[:, 3, k : k + 1],
                            )
                            nc.tensor.matmul(
                                accps[:],
                                lhsT=identb[:],
                                rhs=wx[:],
                                start=(coli == 0),
                                stop=(coli == ncols - 1),
                            )
                            coli += 1
                    # ---- epilogue: agg (already normalized) -> lin0 -> elu -> x1/a1
                    acc_sb = sp0.tile([128, 512], bf16, tag="accsb")
                    nc.scalar.copy(out=acc_sb[:], in_=accps[:])
                    aggT = sp0.tile([128, 512], bf16, tag="aggT")
                    for h in range(4):
                        trp = trp0.tile([128, 128], bf16, tag="tr", name=f"tr0_{g}_{h}")
                        nc.tensor.transpose(
                            out=trp[:],
                            in_=acc_sb[:, 128 * h : 128 * (h + 1)],
                            identity=identb[:],
                        )
                        nc.scalar.copy(
                            out=aggT[:, 128 * h : 128 * (h + 1)], in_=trp[:]
                        )
                    outps = outp0.tile([128, 512], f32, tag="out0")
                    for h in range(4):
                        nc.tensor.matmul(
                            outps[:, 128 * h : 128 * (h + 1)],
                            lhsT=aggT[:, 128 * h : 128 * (h + 1)],
                            rhs=lin0_s[:, 128 * h : 128 * (h + 1)],
                            start=True,
                            stop=True,
                        )
                    h1 = sp0.tile([128, 512], f32, tag="h1")
                    nc.vector.tensor_tensor(out=h1[:], in0=outps[:], in1=bias0r, op=OP.add)
                    u = sp0.tile([128, 512], f32, tag="elu_u")
                    nc.vector.tensor_scalar(
                        out=u[:], in0=h1[:], scalar1=0.0, scalar2=None, op0=OP.min
                    )
                    v = sp0.tile([128, 512], f32, tag="elu_v")
                    nc.scalar.activation(v[:], u[:], AF.Exp)
                    nc.vector.tensor_tensor(out=h1[:], in0=h1[:], in1=u[:], op=OP.subtract)
                    nc.vector.tensor_tensor(out=h1[:], in0=h1[:], in1=v[:], op=OP.add)
                    h1b = sp0.tile([128, 512], bf16, tag="h1b")
                    nc.vector.tensor_scalar(
                        out=h1b[:], in0=h1[:], scalar1=1.0, scalar2=None, op0=OP.subtract
                    )
                    h1T = sp0.tile([128, 512], bf16, tag="h1T")
                    for cc in range(4):
                        trp = trp0.tile([128, 128], bf16, tag="tr", name=f"trh1_{g}_{cc}")
                        nc.tensor.transpose(
                            out=trp[:],
                            in_=h1b[:, 128 * cc : 128 * (cc + 1)],
                            identity=identb[:],
                        )
                        nc.scalar.copy(
                            out=h1T[:, 128 * cc : 128 * (cc + 1)], in_=trp[:]
                        )
                    x1ps = outp0.tile([128, 512], f32, tag="x1")
                    aps = ap0.tile([128, 8], f32, tag="aps")
                    for cc in range(4):
                        nc.tensor.matmul(
                            x1ps[:],
                            lhsT=h1T[:, 128 * cc : 128 * (cc + 1)],
                            rhs=lin1_s[cc][:],
                            start=(cc == 0),
                            stop=(cc == 3),
                        )
                    for cc in range(4):
                        nc.tensor.matmul(
                            aps[:],
                            lhsT=h1T[:, 128 * cc : 128 * (cc + 1)],
                            rhs=wa1_s[cc][:],
                            start=(cc == 0),
                            stop=(cc == 3),
                        )
                    stg = sp0.tile([128, ROWF1], bf16, tag="stg1")
                    nc.scalar.copy(out=stg[:, 0:512], in_=x1ps[:])
                    nc.vector.tensor_copy(out=stg[:, 512:516], in_=aps[:, 0:4])
                    nc.vector.tensor_copy(
                        out=adst1_s[:, 4 * g : 4 * (g + 1)], in_=aps[:, 4:8]
                    )
                    if g == 0:
                        nc.vector.memset(stg[0:1, 512:516], -1e30)
                    nc.sync.dma_start(
                        out=agin[128 * g : 128 * (g + 1), :], in_=stg[:]
                    )

            # ---------------- AllGather table1
            with nc.named_scope("ag"):
                nc.gpsimd.collective_compute(
                    "AllGather",
                    mybir.AluOpType.bypass,
                    replica_groups=[list(range(R))],
                    ins=[agin[:].opt()],
                    outs=[table1[:].opt()],
                )

            # ---------------- L1 aggregation + final MLP (fused per group)
            with nc.named_scope("l1"), tc.tile_pool(name="l1g", bufs=8) as gp1, tc.tile_pool(
                name="l1w", bufs=6
            ) as wp1p, tc.tile_pool(name="l1e", bufs=12) as ep1, tc.tile_pool(
                name="l1t", bufs=4
            ) as tp1, tc.tile_pool(
                name="l1m", bufs=3
            ) as sp1, tc.tile_pool(
                name="l1acc", bufs=3, space="PSUM"
            ) as accp1, tc.tile_pool(
                name="l1z", bufs=2, space="PSUM"
            ) as zp1, tc.tile_pool(
                name="l1tr", bufs=2, space="PSUM"
            ) as trp1:
                for g in range(G):
                    calls = plan_by_group[g]
                    ncols = sum(c for (_, _, c, _) in calls)
                    accps = accp1.tile([128, 512], f32, tag="acc")
                    s_acc = sp1.tile([128, 4], f32, tag="sacc")
                    nc.vector.memset(s_acc[:], 0.0)
                    work = []
                    for ioff, col, c, h in calls:
                        xg = gp1.tile([128, CJ, ROWF1], bf16, tag="xg")
                        gather(
                            xg[:, 0:c, :],
                            table1[BASES[h] : BASES[h] + WIN, :],
                            ioff,
                            c,
                            ROWF1,
                        )
                        et = ep1.tile([128, 4, CJ], f32, tag="et")
                        nc.vector.tensor_tensor(
                            out=et[:, :, 0:c],
                            in0=xg[:, 0:c, 512:516].rearrange("p c f -> p f c"),
                            in1=adst1_s[:, 4 * g : 4 * (g + 1)].to_broadcast(
                                [128, 4, c]
                            ),
                            op=OP.add,
                        )
                        lr = tp1.tile([128, 4, CJ], f32, tag="lr")
                        nc.vector.tensor_scalar(
                            out=lr[:, :, 0:c], in0=et[:, :, 0:c], scalar1=NEG,
                            scalar2=None, op0=OP.mult,
                        )
                        nc.vector.tensor_tensor(
                            out=et[:, :, 0:c], in0=et[:, :, 0:c], in1=lr[:, :, 0:c],
                            op=OP.max,
                        )
                        nc.scalar.activation(et[:, :, 0:c], et[:, :, 0:c], AF.Exp)
                        red = tp1.tile([128, 4], f32, tag="red")
                        nc.vector.tensor_reduce(
                            red[:], et[:, :, 0:c], axis=AX.X, op=OP.add
                        )
                        nc.vector.tensor_tensor(
                            out=s_acc[:], in0=s_acc[:], in1=red[:], op=OP.add
                        )
                        work.append((xg, et, col, c))
                    srec = sp1.tile([128, 4], f32, tag="srec")
                    nc.vector.tensor_scalar(
                        out=srec[:], in0=s_acc[:], scalar1=1e-16, scalar2=None, op0=OP.add
                    )
                    nc.vector.reciprocal(srec[:], srec[:])
                    coli = 0
                    for xg, et, col, c in work:
                        nc.vector.tensor_tensor(
                            out=et[:, :, 0:c],
                            in0=et[:, :, 0:c],
                            in1=ew_s[:, None, col : col + c].to_broadcast([128, 4, c]),
                            op=OP.mult,
                        )
                        nc.vector.tensor_tensor(
                            out=et[:, :, 0:c],
                            in0=et[:, :, 0:c],
                            in1=srec[:, :, None].to_broadcast([128, 4, c]),
                            op=OP.mult,
                        )
                        for k in range(c):
                            wx = wp1p.tile([128, 512], bf16, tag="wx")
                            nc.vector.tensor_tensor(
                                out=wx[:, 0:384].rearrange("p (h f) -> p h f", h=3),
                                in0=xg[:, k, 0:384].rearrange("p (h f) -> p h f", h=3),
                                in1=et[:, 0:3, k].to_broadcast([128, 3, 128]),
                                op=OP.mult,
                            )
                            nc.scalar.activation(
                                wx[:, 384:512],
                                xg[:, k, 384:512],
                                AF.Copy,
                                scale=et[:, 3, k : k + 1],
                            )
                            nc.tensor.matmul(
                                accps[:],
                                lhsT=identb[:],
                                rhs=wx[:],
                                start=(coli == 0),
                                stop=(coli == ncols - 1),
                            )
                            coli += 1
                    h2 = sp1.tile([128, 512], f32, tag="h2")
                    nc.vector.tensor_tensor(out=h2[:], in0=accps[:], in1=bias1r, op=OP.add)
                    u = sp1.tile([128, 512], f32, tag="elu_u")
                    nc.vector.tensor_scalar(
                        out=u[:], in0=h2[:], scalar1=0.0, scalar2=None, op0=OP.min
                    )
                    v = sp1.tile([128, 512], f32, tag="elu_v")
                    nc.scalar.activation(v[:], u[:], AF.Exp)
                    nc.vector.tensor_tensor(out=h2[:], in0=h2[:], in1=u[:], op=OP.subtract)
                    nc.vector.tensor_tensor(out=h2[:], in0=h2[:], in1=v[:], op=OP.add)
                    h2b = sp1.tile([128, 512], bf16, tag="h2b")
                    nc.vector.tensor_scalar(
                        out=h2b[:], in0=h2[:], scalar1=1.0, scalar2=None, op0=OP.subtract
                    )
                    hT = sp1.tile([128, 640], bf16, tag="hT")
                    for cc in range(4):
                        trp = trp1.tile([128, 128], bf16, tag="tr", name=f"trh2_{g}_{cc}")
                        nc.tensor.transpose(
                            out=trp[:],
                            in_=h2b[:, 128 * cc : 128 * (cc + 1)],
                            identity=identb[:],
                        )
                        nc.scalar.copy(
                            out=hT[:, 128 * cc : 128 * (cc + 1)], in_=trp[:]
                        )
                    hrb = sp1.tile([128, 128], bf16, tag="hrb")
                    nc.sync.dma_start(out=hrb[:], in_=h_res_d[g, :, :])
                    trp = trp1.tile([128, 128], bf16, tag="tr", name=f"trhr_{g}")
                    nc.tensor.transpose(out=trp[:], in_=hrb[:], identity=identb[:])
                    nc.scalar.copy(out=hT[:, 512:640], in_=trp[:])
                    zps = zp1.tile([128, 128], f32, tag="z")
                    for cc in range(5):
                        nc.tensor.matmul(
                            zps[:],
                            lhsT=hT[:, 128 * cc : 128 * (cc + 1)],
                            rhs=wp1_s[cc][:],
                            start=(cc == 0),
                            stop=(cc == 4),
                        )
                    zsb = sp1.tile([128, 128], f32, tag="zsb")
                    nc.scalar.copy(out=zsb[:], in_=zps[:])
                    nc.vector.tensor_tensor(out=zsb[:], in0=zsb[:], in1=bp1r, op=OP.add)
                    nc.scalar.activation(zsb[:], zsb[:], AF.Gelu_apprx_tanh)
                    nc.vector.tensor_tensor(out=zsb[:], in0=zsb[:], in1=wp2r, op=OP.mult)
                    yred = sp1.tile([128, 1], f32, tag="yred")
                    nc.vector.tensor_reduce(yred[:], zsb[:], axis=AX.X, op=OP.add)
                    nc.vector.tensor_scalar(
                        out=ysb[:, g : g + 1],
                        in0=yred[:],
                        scalar1=bp2t[:, 0:1],
                        scalar2=None,
                        op0=OP.add,
                    )
            nc.sync.dma_start(out=y_d[:, :], in_=ysb[:])

    nc.compile()
    return nc


_CACHE = {}


def kernel(
    X,
    edge_index,
    edge_weight,
    W_in,
    b_in,
    lin0,
    att_src0,
    att_dst0,
    bias0,
    lin1,
    att_src1,
    att_dst1,
    bias1,
    Wp1,
    bp1,
    Wp2,
    bp2,
):
    import ml_dtypes
    from concourse.bass_utils import run_bass_kernel_spmd

    bfloat16 = ml_dtypes.bfloat16

    X = np.asarray(X, np.float32)
    perm, gidx, ewt, idx16, meta = _prep_graph(
        np.asarray(edge_index), np.asarray(edge_weight)
    )
    rhs0, rhs0l, wa1 = _prep_weights(
        np.asarray(W_in, np.float32),
        np.asarray(b_in, np.float32),
        np.asarray(lin0, np.float32),
        np.asarray(att_src0, np.float32),
        np.asarray(att_dst0, np.float32),
        np.asarray(lin1, np.float32),
        np.asarray(att_src1, np.float32),
        np.asarray(att_dst1, np.float32),
    )
    xptb = _build_xpt_blocks(X, perm)

    key = ("nc", meta["tot_cols"], tuple(meta["call_plan"]))
    if key not in _CACHE:
        _CACHE[key] = _build_nc(meta["call_plan"], meta["tot_cols"])
    nc = _CACHE[key]

    brow = np.zeros((1, 1280), np.float32)
    brow[0, 0:512] = np.asarray(bias0, np.float32)
    brow[0, 512:1024] = np.asarray(bias1, np.float32)
    brow[0, 1024:1152] = np.asarray(bp1, np.float32)
    brow[0, 1152:1280] = np.asarray(Wp2, np.float32)[:, 0]

    common = dict(
        xptb=xptb.astype(bfloat16),
        rhs0=rhs0.astype(bfloat16),
        rhs0l=rhs0l.astype(bfloat16),
        lin0=np.ascontiguousarray(np.asarray(lin0, np.float32)).astype(bfloat16),
        lin1=np.ascontiguousarray(np.asarray(lin1, np.float32)).astype(bfloat16),
        wa1=wa1.astype(bfloat16),
        wp1=np.ascontiguousarray(np.asarray(Wp1, np.float32)).astype(bfloat16),
        brow=brow,
        bp2=np.asarray(bp2, np.float32).reshape(1, 1),
    )
    in_maps = [
        dict(common, xptl=xptb[G * r : G * (r + 1)].astype(bfloat16), idx16=idx16[r], ewt=ewt[r])
        for r in range(R)
    ]

    res = run_bass_kernel_spmd(nc, in_maps, core_ids=list(range(R)), trace=False)

    y = np.zeros(N, np.float32)
    for r in range(R):
        yflat = res.results[r]["y"].T.reshape(-1)
        y[perm[r]] = yflat[DUM : DUM + perm[r].shape[0]]
    return y


# revision 15
# speedup vs baseline: 1.0046x; 1.0046x over previous
"""Trainium2 Bass kernel for nn_GATPredictor (2-layer multi-head GAT + MLP).

kernel(**inputs) takes FULL unsharded numpy inputs, returns the FULL
(50000,) float32 output.  Internally: 8-way dst-node sharding (degree
round-robin), replicated weights, one AllGather of layer-1 src features,
padded per-(group,half) neighbor-slot gathers via dma_gather (int16 idx,
two half-tables), two-pass per-group softmax with the normalization
folded into the edge weights, per-edge weighting split 3 heads on DVE +
1 head on ACT, and PSUM-accumulated identity matmuls (bf16) for the
weighted scatter-add.
"""

import numpy as np

N = 50000
E = 800000
F_IN = 64
H = 4
C = 128
NEG = 0.2
R = 8
PER = N // R
DUM = 22
CHUNK = PER + DUM  # 6272
G = CHUNK // 128  # 49
NPOS = R * CHUNK  # 50176
HALF = NPOS // 2  # 25088
CJ = 12
NB = NPOS // 128  # 392
WIN = 32768
BASES = [0, (NPOS - WIN) // 2, NPOS - WIN]  # [0, 8704, 17408]
NW = 3
# per-window pad index: first dummy (rank-chunk start) row inside the window
PADIDX = [min(x for x in range(0, NPOS, CHUNK) if x >= b) - b for b in BASES]

ROWF0 = 256  # table0 row (bf16): [h(128) | asrc(4) | pad] -> 512B
ROWF1 = 640  # table1 row (bf16): [x1(512) | asrc(4) | pad] -> 1280B


# ---------------------------------------------------------------- host prep
def _prep_graph(edge_index, edge_weight):
    src = np.concatenate([np.asarray(edge_index[0], np.int64), np.arange(N)])
    dst = np.concatenate([np.asarray(edge_index[1], np.int64), np.arange(N)])
    ew = np.concatenate([np.asarray(edge_weight, np.float32), np.ones(N, np.float32)])

    deg = np.bincount(dst, minlength=N)
    gorder = np.argsort(deg, kind="stable")
    rank_of_node = np.empty(N, np.int64)
    rank_of_node[gorder] = np.arange(N) % R

    pos_of_node = np.empty(N, np.int64)
    perm_per_rank = []
    for r in range(R):
        nodes = np.where(rank_of_node == r)[0]
        order = np.argsort(deg[nodes], kind="stable")
        sorted_nodes = nodes[order]
        perm_per_rank.append(sorted_nodes)
        pos_of_node[sorted_nodes] = CHUNK * r + DUM + np.arange(nodes.shape[0])

    src_pos = pos_of_node[src]
    dst_pos = pos_of_node[dst]

    basesv = np.asarray(BASES)
    lo = np.searchsorted(basesv + WIN, src_pos, side="right")
    hi = np.searchsorted(basesv, src_pos, side="right") - 1
    assert (lo <= hi).all() and (lo >= 0).all() and (hi < NW).all()

    # per-lane rank in pos order within each dst
    order = np.lexsort((src_pos, dst_pos))
    dpo, loo, hio, spo = dst_pos[order], lo[order], hi[order], src_pos[order]
    swo = ew[order]
    tot = np.bincount(dst_pos, minlength=NPOS)
    change = np.ones(dpo.shape[0], np.bool_)
    change[1:] = dpo[1:] != dpo[:-1]
    run_start = np.maximum.accumulate(np.where(change, np.arange(dpo.shape[0]), 0))
    k_in = np.arange(dpo.shape[0]) - run_start

    # Hall capacities per group
    lanes = lambda cnt: cnt.reshape(R, G, 128)
    H0 = lanes(np.bincount(dpo[hio <= 0], minlength=NPOS)).max(axis=(0, 2))
    H1 = lanes(np.bincount(dpo[hio <= 1], minlength=NPOS)).max(axis=(0, 2))
    L2 = lanes(np.bincount(dpo[loo >= 2], minlength=NPOS)).max(axis=(0, 2))
    L1 = lanes(np.bincount(dpo[loo >= 1], minlength=NPOS)).max(axis=(0, 2))
    M1 = lanes(np.bincount(dpo[(loo == 1) & (hio == 1)], minlength=NPOS)).max(axis=(0, 2))
    Tm = lanes(tot).max(axis=(0, 2))
    C0 = H0
    C2 = L2
    C1 = np.maximum.reduce([H1 - C0, L1 - C2, Tm - C0 - C2, M1, np.zeros(G, np.int64)])

    # earliest-fit per lane: window w takes (in pos order) up to C_w of the
    # not-yet-assigned edges with lo <= w
    g_of = (dpo % CHUNK) // 128

    def masked_rank(mask):
        # rank of each True element within its (dst) run, counting only True
        cm = np.cumsum(mask)
        base = np.zeros_like(cm)
        base[run_start] = cm[run_start] - mask[run_start]
        base = np.maximum.accumulate(np.where(change, base, 0))
        return cm - 1 - base

    wsel = np.full(dpo.shape[0], NW - 1, np.int64)
    unassigned = np.ones(dpo.shape[0], np.bool_)
    caps = [C0, C1]
    for w in range(NW - 1):
        elig = unassigned & (loo <= w)
        rk = masked_rank(elig)
        take = elig & (rk < caps[w][g_of])
        wsel[take] = w
        unassigned &= ~take
    assert (wsel >= loo).all() and (wsel <= hio).all(), "window assignment infeasible" 

    # actual per-(lane,window) loads and slot within (dst, window)
    loads = np.zeros((NPOS, NW), np.int64)
    np.add.at(loads, (dpo, wsel), 1)
    Duni = loads.reshape(R, G, 128, NW).max(axis=(0, 2))  # (G, NW)

    call_plan = []
    for g in range(G):
        for w in range(NW):
            left = int(Duni[g, w])
            while left > 0:
                c = min(CJ, left)
                call_plan.append((g, w, c))
                left -= c
    tot_cols = int(Duni.sum())

    colbase = np.zeros((G, NW), np.int64)
    acc = 0
    for g in range(G):
        for w in range(NW):
            colbase[g, w] = acc
            acc += int(Duni[g, w])
    assert acc == tot_cols

    # slot index within (dst, window): order edges by (dst, window, pos)
    order2 = np.lexsort((spo, wsel, dpo))
    sd, sw_, si_pos, swt = dpo[order2], wsel[order2], spo[order2], swo[order2]
    si = si_pos - basesv[sw_]
    assert (si >= 0).all() and (si < WIN).all()
    key = sd * NW + sw_
    change = np.ones(key.shape[0], np.bool_)
    change[1:] = key[1:] != key[:-1]
    run_start = np.maximum.accumulate(np.where(change, np.arange(key.shape[0]), 0))
    slot_k = np.arange(key.shape[0]) - run_start

    e_rank = sd // CHUNK
    e_loc = sd - e_rank * CHUNK
    e_g = e_loc // 128
    e_p = e_loc % 128
    e_col = colbase[e_g, sw_] + slot_k

    gidx = np.zeros((R, 128, tot_cols), np.int32)
    ewt = np.zeros((R, 128, tot_cols), np.float32)
    for g in range(G):
        for w in range(NW):
            gidx[:, :, colbase[g, w] : colbase[g, w] + Duni[g, w]] = PADIDX[w]
    gidx[e_rank, e_p, e_col] = si
    ewt[e_rank, e_p, e_col] = swt

    idx16 = np.zeros((R, 128, 8 * tot_cols), np.int16)
    cursor = {(g, w): 0 for g in range(G) for w in range(NW)}
    off = 0
    for g, w, c in call_plan:
        cb = colbase[g, w] + cursor[(g, w)]
        cursor[(g, w)] += c
        cols = gidx[:, :, cb : cb + c]  # (R, 128, c)
        stream = cols.transpose(0, 2, 1).reshape(R, -1)
        wv = stream.reshape(R, c * 8, 16).transpose(0, 2, 1).astype(np.int16)
        idx16[:, :, off : off + 8 * c] = np.tile(wv, (1, 8, 1))
        off += 8 * c
    assert off == 8 * tot_cols

    meta = dict(call_plan=call_plan, tot_cols=tot_cols)
    return perm_per_rank, gidx, ewt, idx16, meta


def _prep_weights(W_in, b_in, lin0, a_src0, a_dst0, lin1, a_src1, a_dst1):
    def fold(lin, a):
        return np.einsum(
            "ihc,hc->ih",
            lin.reshape(lin.shape[0], H, C).astype(np.float64),
            a.astype(np.float64),
        ).astype(np.float32)

    w_src0, w_dst0 = fold(lin0, a_src0), fold(lin0, a_dst0)
    rhs0 = np.zeros((128, 136), np.float32)
    rhs0[:F_IN, :C] = W_in
    rhs0[:F_IN, C : C + 4] = W_in @ w_src0
    rhs0[:F_IN, C + 4 :] = W_in @ w_dst0
    rhs0[F_IN, :C] = b_in
    rhs0[F_IN, C : C + 4] = b_in @ w_src0
    rhs0[F_IN, C + 4 :] = b_in @ w_dst0
    rhs0L = np.zeros((128, 132), np.float32)
    rhs0L[:F_IN, :C] = W_in
    rhs0L[:F_IN, C:] = W_in @ w_dst0
    rhs0L[F_IN, :C] = b_in
    rhs0L[F_IN, C:] = b_in @ w_dst0
    wa1 = np.concatenate([fold(lin1, a_src1), fold(lin1, a_dst1)], axis=1)
    return rhs0, rhs0L, wa1.astype(np.float32)


def _build_xpt_blocks(X, perm_per_rank):
    xpt = np.zeros((128, NPOS), np.float32)
    xpt[F_IN, :] = 1.0
    for r in range(R):
        cols = CHUNK * r + DUM + np.arange(perm_per_rank[r].shape[0])
        xpt[:F_IN, cols] = X[perm_per_rank[r]].T
    return xpt.reshape(128, NB, 128).transpose(1, 0, 2).copy()


# ---------------------------------------------------------------- bass build
def _build_nc(call_plan, tot_cols):
    import concourse.bacc as bacc
    import concourse.bass_isa as bass_isa
    import concourse.mybir as mybir
    import concourse.tile as tile
    from concourse.masks import make_identity

    f32 = mybir.dt.float32
    bf16 = mybir.dt.bfloat16
    i16 = mybir.dt.int16
    AF = mybir.ActivationFunctionType
    OP = mybir.AluOpType
    AX = mybir.AxisListType

    nc = bacc.Bacc(
        "TRN2",
        target_bir_lowering=False,
        debug=False,
        enable_asserts=False,
        num_devices=R,
        num_swdge_queues=4,
    )

    xptb = nc.dram_tensor("xptb", [NB, 128, 128], f32, kind="ExternalInput")
    xptl = nc.dram_tensor("xptl", [G, 128, 128], f32, kind="ExternalInput")
    rhs0_d = nc.dram_tensor("rhs0", [128, 136], f32, kind="ExternalInput")
    rhs0l_d = nc.dram_tensor("rhs0l", [128, 132], f32, kind="ExternalInput")
    lin0_d = nc.dram_tensor("lin0", [128, 512], bf16, kind="ExternalInput")
    lin1_d = nc.dram_tensor("lin1", [512, 512], bf16, kind="ExternalInput")
    wa1_d = nc.dram_tensor("wa1", [512, 8], bf16, kind="ExternalInput")
    wp1_d = nc.dram_tensor("wp1", [640, 128], bf16, kind="ExternalInput")
    brow_d = nc.dram_tensor("brow", [1, 1280], f32, kind="ExternalInput")
    bp2_d = nc.dram_tensor("bp2", [1, 1], f32, kind="ExternalInput")
    idx_d = nc.dram_tensor("idx16", [128, 8 * tot_cols], i16, kind="ExternalInput")
    ew_d = nc.dram_tensor("ewt", [128, tot_cols], f32, kind="ExternalInput")
    y_d = nc.dram_tensor("y", [128, G], f32, kind="ExternalOutput")

    # group -> list of (idx_off8, col, ncols, half); order mirrors host prep
    plan_by_group = [[] for _ in range(G)]
    colbase = {}
    acc = 0
    for g in range(G):
        for h in range(NW):
            cols_gh = sum(c for (gg, hh, c) in call_plan if gg == g and hh == h)
            colbase[(g, h)] = acc
            acc += cols_gh
    cursor = {(g, h): 0 for g in range(G) for h in range(NW)}
    off8 = 0
    for g, h, c in call_plan:
        col = colbase[(g, h)] + cursor[(g, h)]
        cursor[(g, h)] += c
        plan_by_group[g].append((off8, col, c, h))
        off8 += 8 * c

    with tile.TileContext(nc) as tc:
        with tc.tile_pool(name="dram", bufs=1, space="DRAM") as dram, tc.tile_pool(
            name="const", bufs=1
        ) as cp:
            table0 = dram.tile([NPOS, ROWF0], bf16)
            h_res_d = dram.tile([G, 128, 128], bf16)
            agin = dram.tile([CHUNK, ROWF1], bf16)
            table1 = dram.tile([NPOS, ROWF1], bf16)

            ident = cp.tile([128, 128], f32)
            make_identity(nc, ident[:])
            identb = cp.tile([128, 128], bf16)
            nc.vector.tensor_copy(out=identb[:], in_=ident[:])
            rhs0_s = cp.tile([128, 136], f32)
            nc.sync.dma_start(out=rhs0_s[:], in_=rhs0_d[:, :])
            rhs0l_s = cp.tile([128, 132], f32)
            nc.sync.dma_start(out=rhs0l_s[:], in_=rhs0l_d[:, :])
            lin0_s = cp.tile([128, 512], bf16)
            nc.sync.dma_start(out=lin0_s[:], in_=lin0_d[:, :])
            lin1_s = [cp.tile([128, 512], bf16, tag=f"lin1_{c}", name=f"lin1s{c}") for c in range(4)]
            wa1_s = [cp.tile([128, 8], bf16, tag=f"wa1_{c}", name=f"wa1s{c}") for c in range(4)]
            for c in range(4):
                nc.sync.dma_start(
                    out=lin1_s[c][:], in_=lin1_d[128 * c : 128 * (c + 1), :]
                )
                nc.sync.dma_start(
                    out=wa1_s[c][:], in_=wa1_d[128 * c : 128 * (c + 1), :]
                )
            wp1_s = [cp.tile([128, 128], bf16, tag=f"wp1_{c}", name=f"wp1s{c}") for c in range(5)]
            for c in range(5):
                nc.sync.dma_start(
                    out=wp1_s[c][:], in_=wp1_d[128 * c : 128 * (c + 1), :]
                )
            brow = cp.tile([128, 1280], f32)
            nc.sync.dma_start(out=brow[0:1, :], in_=brow_d[:, :])
            nc.gpsimd.partition_broadcast(brow[:], brow[0:1, :])
            bias0r = brow[:, 0:512]
            bias1r = brow[:, 512:1024]
            bp1r = brow[:, 1024:1152]
            wp2r = brow[:, 1152:1280]
            bp2t = cp.tile([128, 1], f32)
            nc.sync.dma_start(out=bp2t[0:1, :], in_=bp2_d[:, :])
            nc.gpsimd.partition_broadcast(bp2t[:], bp2t[0:1, :])
            idx_s = cp.tile([128, 8 * tot_cols], i16)
            nc.sync.dma_start(out=idx_s[:], in_=idx_d[:, :])
            ew_s = cp.tile([128, tot_cols], f32)
            nc.sync.dma_start(out=ew_s[:], in_=ew_d[:, :])
            adst0_s = cp.tile([128, G * 4], f32)
            adst1_s = cp.tile([128, G * 4], f32)
            ysb = cp.tile([128, G], f32)

            # ---------------- M0: replicated table0 + M0L local
            with nc.named_scope("m0"), tc.tile_pool(name="m0s", bufs=3) as mp, tc.tile_pool(
                name="m0p", bufs=2, space="PSUM"
            ) as mpp:
                for b in range(NB):
                    xb = mp.tile([128, 128], f32, tag="xb")
                    nc.sync.dma_start(out=xb[:], in_=xptb[b, :, :])
                    ps = mpp.tile([128, 136], f32, tag="m0ps")
                    nc.tensor.matmul(
                        ps[:], lhsT=xb[:], rhs=rhs0_s[:], start=True, stop=True
                    )
                    stg = mp.tile([128, ROWF0], bf16, tag="stg0")
                    nc.scalar.copy(out=stg[:, 0:132], in_=ps[:, 0:132])
                    if b % G == 0:
                        nc.vector.memset(stg[0:1, 128:132], -1e30)
                    nc.sync.dma_start(
                        out=table0[128 * b : 128 * (b + 1), :], in_=stg[:]
                    )
                for g in range(G):
                    xb = mp.tile([128, 128], f32, tag="xb")
                    nc.sync.dma_start(out=xb[:], in_=xptl[g, :, :])
                    ps = mpp.tile([128, 136], f32, tag="m0ps")
                    nc.tensor.matmul(
                        ps[:, 0:132], lhsT=xb[:], rhs=rhs0l_s[:], start=True, stop=True
                    )
                    stg = mp.tile([128, 128], bf16, tag="stgL")
                    nc.scalar.copy(out=stg[:], in_=ps[:, 0:128])
                    nc.sync.dma_start(out=h_res_d[g, :, :], in_=stg[:])
                    nc.vector.tensor_copy(
                        out=adst0_s[:, 4 * g : 4 * (g + 1)], in_=ps[:, 128:132]
                    )

            qn = [0]

            def gather(out_ap, half_ap, ioff, c, elem):
                nc.gpsimd.dma_gather(
                    out_ap=out_ap,
                    in_ap=half_ap,
                    idxs_ap=idx_s[:, ioff : ioff + 8 * c],
                    num_idxs=128 * c,
                    num_idxs_reg=128 * c,
                    elem_size=elem,
                    single_packet=False,
                    queue_num=qn[0] % 4,
                )
                qn[0] += 1

            # ---------------- L0 aggregation + M2 (fused per group)
            with nc.named_scope("l0"), tc.tile_pool(name="l0g", bufs=12) as gp0, tc.tile_pool(
                name="l0w", bufs=6
            ) as wp0, tc.tile_pool(name="l0e", bufs=12) as ep0, tc.tile_pool(
                name="l0t", bufs=4
            ) as tp0, tc.tile_pool(
                name="l0m", bufs=3
            ) as sp0, tc.tile_pool(
                name="l0acc", bufs=3, space="PSUM"
            ) as accp0, tc.tile_pool(
                name="l0out", bufs=1, space="PSUM"
            ) as outp0, tc.tile_pool(
                name="l0a", bufs=1, space="PSUM"
            ) as ap0, tc.tile_pool(
                name="l0tr", bufs=2, space="PSUM"
            ) as trp0:
                for g in range(G):
                    calls = plan_by_group[g]
                    ncols = sum(c for (_, _, c, _) in calls)
                    accps = accp0.tile([128, 512], f32, tag="acc")
                    s_acc = sp0.tile([128, 4], f32, tag="sacc")
                    nc.vector.memset(s_acc[:], 0.0)
                    work = []
                    # ---- pass 1: gather + alpha(exp) + s accumulation
                    for ioff, col, c, h in calls:
                        xg = gp0.tile([128, CJ, ROWF0], bf16, tag="xg")
                        gather(
                            xg[:, 0:c, :],
                            table0[BASES[h] : BASES[h] + WIN, :],
                            ioff,
                            c,
                            ROWF0,
                        )
                        et = ep0.tile([128, 4, CJ], f32, tag="et")
                        nc.vector.tensor_tensor(
                            out=et[:, :, 0:c],
                            in0=xg[:, 0:c, 128:132].rearrange("p c f -> p f c"),
                            in1=adst0_s[:, 4 * g : 4 * (g + 1)].to_broadcast(
                                [128, 4, c]
                            ),
                            op=OP.add,
                        )
                        lr = tp0.tile([128, 4, CJ], f32, tag="lr")
                        nc.vector.tensor_scalar(
                            out=lr[:, :, 0:c], in0=et[:, :, 0:c], scalar1=NEG,
                            scalar2=None, op0=OP.mult,
                        )
                        nc.vector.tensor_tensor(
                            out=et[:, :, 0:c], in0=et[:, :, 0:c], in1=lr[:, :, 0:c],
                            op=OP.max,
                        )
                        nc.scalar.activation(et[:, :, 0:c], et[:, :, 0:c], AF.Exp)
                        red = tp0.tile([128, 4], f32, tag="red")
                        nc.vector.tensor_reduce(
                            red[:], et[:, :, 0:c], axis=AX.X, op=OP.add
                        )
                        nc.vector.tensor_tensor(
                            out=s_acc[:], in0=s_acc[:], in1=red[:], op=OP.add
                        )
                        work.append((xg, et, col, c))
                    # ---- normalization folded into weights
                    srec = sp0.tile([128, 4], f32, tag="srec")
                    nc.vector.tensor_scalar(
                        out=srec[:], in0=s_acc[:], scalar1=1e-16, scalar2=None, op0=OP.add
                    )
                    nc.vector.reciprocal(srec[:], srec[:])
                    # ---- pass 2: wt = exp * ew * srec; weighted scatter-add
                    coli = 0
                    for xg, et, col, c in work:
                        nc.vector.tensor_tensor(
                            out=et[:, :, 0:c],
                            in0=et[:, :, 0:c],
                            in1=ew_s[:, None, col : col + c].to_broadcast([128, 4, c]),
                            op=OP.mult,
                        )
                        nc.vector.tensor_tensor(
                            out=et[:, :, 0:c],
                            in0=et[:, :, 0:c],
                            in1=srec[:, :, None].to_broadcast([128, 4, c]),
                            op=OP.mult,
                        )
                        for k in range(c):
                            wx = wp0.tile([128, 512], bf16, tag="wx")
                            nc.vector.tensor_tensor(
                                out=wx[:, 0:384].rearrange("p (h f) -> p h f", h=3),
                                in0=xg[:, k, 0:128][:, None, :].to_broadcast(
                                    [128, 3, 128]
                                ),
                                in1=et[:, 0:3, k].to_broadcast([128, 3, 128]),
                                op=OP.mult,
                            )
                            nc.scalar.activation(
                                wx[:, 384:512],
                                xg[:, k, 0:128],
                                AF.Copy,
                                scale=et[:, 3, k : k + 1],
                            )
                            nc.tensor.matmul(
                                accps[:],
                                lhsT=identb[:],
                                rhs=wx[:],
                                start=(coli == 0),
                                stop=(coli == ncols - 1),
                            )
                            coli += 1
                    # ---- epilogue: agg (already normalized) -> lin0 -> elu -> x1/a1
                    acc_sb = sp0.tile([128, 512], bf16, tag="accsb")
                    nc.scalar.copy(out=acc_sb[:], in_=accps[:])
                    aggT = sp0.tile([128, 512], bf16, tag="aggT")
                    for h in range(4):
                        trp = trp0.tile([128, 128], bf16, tag="tr", name=f"tr0_{g}_{h}")
                        nc.tensor.transpose(
                            out=trp[:],
                            in_=acc_sb[:, 128 * h : 128 * (h + 1)],
                            identity=identb[:],
                        )
                        nc.scalar.copy(
                            out=aggT[:, 128 * h : 128 * (h + 1)], in_=trp[:]
                        )
                    outps = outp0.tile([128, 512], f32, tag="out0")
                    for h in range(4):
                        nc.tensor.matmul(
                            outps[:, 128 * h : 128 * (h + 1)],
                            lhsT=aggT[:, 128 * h : 128 * (h + 1)],
                            rhs=lin0_s[:, 128 * h : 128 * (h + 1)],
                            start=True,
                            stop=True,
                        )
                    h1 = sp0.tile([128, 512], f32, tag="h1")
                    nc.vector.tensor_tensor(out=h1[:], in0=outps[:], in1=bias0r, op=OP.add)
                    u = sp0.tile([128, 512], f32, tag="elu_u")
                    nc.vector.tensor_scalar(
                        out=u[:], in0=h1[:], scalar1=0.0, scalar2=None, op0=OP.min
                    )
                    v = sp0.tile([128, 512], f32, tag="elu_v")
                    nc.scalar.activation(v[:], u[:], AF.Exp)
                    nc.vector.tensor_tensor(out=h1[:], in0=h1[:], in1=u[:], op=OP.subtract)
                    nc.vector.tensor_tensor(out=h1[:], in0=h1[:], in1=v[:], op=OP.add)
                    h1b = sp0.tile([128, 512], bf16, tag="h1b")
                    nc.vector.tensor_scalar(
                        out=h1b[:], in0=h1[:], scalar1=1.0, scalar2=None, op0=OP.subtract
                    )
                    h1T = sp0.tile([128, 512], bf16, tag="h1T")
                    for cc in range(4):
                        trp = trp0.tile([128, 128], bf16, tag="tr", name=f"trh1_{g}_{cc}")
                        nc.tensor.transpose(
                            out=trp[:],
                            in_=h1b[:, 128 * cc : 128 * (cc + 1)],
                            identity=identb[:],
                        )
                        nc.scalar.copy(
                            out=h1T[:, 128 * cc : 128 * (cc + 1)], in_=trp[:]
                        )
                    x1ps = outp0.tile([128, 512], f32, tag="x1")
                    aps = ap0.tile([128, 8], f32, tag="aps")
                    for cc in range(4):
                        nc.tensor.matmul(
                            x1ps[:],
                            lhsT=h1T[:, 128 * cc : 128 * (cc + 1)],
                            rhs=lin1_s[cc][:],
                            start=(cc == 0),
                            stop=(cc == 3),
                        )
                    for cc in range(4):
                        nc.tensor.matmul(
                            aps[:],
                            lhsT=h1T[:, 128 * cc : 128 * (cc + 1)],
                            rhs=wa1_s[cc][:],
                            start=(cc == 0),
                            stop=(cc == 3),
                        )
                    stg = sp0.tile([128, ROWF1], bf16, tag="stg1")
                    nc.scalar.copy(out=stg[:, 0:512], in_=x1ps[:])
                    nc.vector.tensor_copy(out=stg[:, 512:516], in_=aps[:, 0:4])
                    nc.vector.tensor_copy(
                        out=adst1_s[:, 4 * g : 4 * (g + 1)], in_=aps[:, 4:8]
                    )
                    if g == 0:
                        nc.vector.memset(stg[0:1, 512:516], -1e30)
                    nc.sync.dma_start(
                        out=agin[128 * g : 128 * (g + 1), :], in_=stg[:]
                    )

            # ---------------- AllGather table1
            with nc.named_scope("ag"):
                nc.gpsimd.collective_compute(
                    "AllGather",
                    mybir.AluOpType.bypass,
                    replica_groups=[list(range(R))],
                    ins=[agin[:].opt()],
                    outs=[table1[:].opt()],
                )

            # ---------------- L1 aggregation + final MLP (fused per group)
            with nc.named_scope("l1"), tc.tile_pool(name="l1g", bufs=8) as gp1, tc.tile_pool(
                name="l1w", bufs=6
            ) as wp1p, tc.tile_pool(name="l1e", bufs=12) as ep1, tc.tile_pool(
                name="l1t", bufs=4
            ) as tp1, tc.tile_pool(
                name="l1m", bufs=3
            ) as sp1, tc.tile_pool(
                name="l1acc", bufs=3, space="PSUM"
            ) as accp1, tc.tile_pool(
                name="l1z", bufs=2, space="PSUM"
            ) as zp1, tc.tile_pool(
                name="l1tr", bufs=2, space="PSUM"
            ) as trp1:
                for g in range(G):
                    calls = plan_by_group[g]
                    ncols = sum(c for (_, _, c, _) in calls)
                    accps = accp1.tile([128, 512], f32, tag="acc")
                    s_acc = sp1.tile([128, 4], f32, tag="sacc")
                    nc.vector.memset(s_acc[:], 0.0)
                    work = []
                    for ioff, col, c, h in calls:
                        xg = gp1.tile([128, CJ, ROWF1], bf16, tag="xg")
                        gather(
                            xg[:, 0:c, :],
                            table1[BASES[h] : BASES[h] + WIN, :],
                            ioff,
                            c,
                            ROWF1,
                        )
                        et = ep1.tile([128, 4, CJ], f32, tag="et")
                        nc.vector.tensor_tensor(
                            out=et[:, :, 0:c],
                            in0=xg[:, 0:c, 512:516].rearrange("p c f -> p f c"),
                            in1=adst1_s[:, 4 * g : 4 * (g + 1)].to_broadcast(
                                [128, 4, c]
                            ),
                            op=OP.add,
                        )
                        lr = tp1.tile([128, 4, CJ], f32, tag="lr")
                        nc.vector.tensor_scalar(
                            out=lr[:, :, 0:c], in0=et[:, :, 0:c], scalar1=NEG,
                            scalar2=None, op0=OP.mult,
                        )
                        nc.vector.tensor_tensor(
                            out=et[:, :, 0:c], in0=et[:, :, 0:c], in1=lr[:, :, 0:c],
                            op=OP.max,
                        )
                        nc.scalar.activation(et[:, :, 0:c], et[:, :, 0:c], AF.Exp)
                        red = tp1.tile([128, 4], f32, tag="red")
                        nc.vector.tensor_reduce(
                            red[:], et[:, :, 0:c], axis=AX.X, op=OP.add
                        )
                        nc.vector.tensor_tensor(
                            out=s_acc[:], in0=s_acc[:], in1=red[:], op=OP.add
                        )
                        work.append((xg, et, col, c))
                    srec = sp1.tile([128, 4], f32, tag="srec")
                    nc.vector.tensor_scalar(
                        out=srec[:], in0=s_acc[:], scalar1=1e-16, scalar2=None, op0=OP.add
                    )
                    nc.vector.reciprocal(srec[:], srec[:])
                    coli = 0
                    for xg, et, col, c in work:
                        nc.vector.tensor_tensor(
                            out=et[:, :, 0:c],
                            in0=et[:, :, 0:c],
                            in1=ew_s[:, None, col : col + c].to_broadcast([128, 4, c]),
                            op=OP.mult,
                        )
                        nc.vector.tensor_tensor(
                            out=et[:, :, 0:c],
                            in0=et[:, :, 0:c],
                            in1=srec[:, :, None].to_broadcast([128, 4, c]),
                            op=OP.mult,
                        )
                        for k in range(c):
                            wx = wp1p.tile([128, 512], bf16, tag="wx")
                            nc.vector.tensor_tensor(
                                out=wx[:, 0:384].rearrange("p (h f) -> p h f", h=3),
                                in0=xg[:, k, 0:384].rearrange("p (h f) -> p h f", h=3),
                                in1=et[:, 0:3, k].to_broadcast([128, 3, 128]),
                                op=OP.mult,
                            )
                            nc.scalar.activation(
                                wx[:, 384:512],
                                xg[:, k, 384:512],
                                AF.Copy,
                                scale=et[:, 3, k : k + 1],
                            )
                            nc.tensor.matmul(
                                accps[:],
                                lhsT=identb[:],
                                rhs=wx[:],
                                start=(coli == 0),
                                stop=(coli == ncols - 1),
                            )
                            coli += 1
                    h2 = sp1.tile([128, 512], f32, tag="h2")
                    nc.vector.tensor_tensor(out=h2[:], in0=accps[:], in1=bias1r, op=OP.add)
                    u = sp1.tile([128, 512], f32, tag="elu_u")
                    nc.vector.tensor_scalar(
                        out=u[:], in0=h2[:], scalar1=0.0, scalar2=None, op0=OP.min
                    )
                    v = sp1.tile([128, 512], f32, tag="elu_v")
                    nc.scalar.activation(v[:], u[:], AF.Exp)
                    nc.vector.tensor_tensor(out=h2[:], in0=h2[:], in1=u[:], op=OP.subtract)
                    nc.vector.tensor_tensor(out=h2[:], in0=h2[:], in1=v[:], op=OP.add)
                    h2b = sp1.tile([128, 512], bf16, tag="h2b")
                    nc.vector.tensor_scalar(
                        out=h2b[:], in0=h2[:], scalar1=1.0, scalar2=None, op0=OP.subtract
                    )
                    hT = sp1.tile([128, 640], bf16, tag="hT")
                    for cc in range(4):
                        trp = trp1.tile([128, 128], bf16, tag="tr", name=f"trh2_{g}_{cc}")
                        nc.tensor.transpose(
                            out=trp[:],
                            in_=h2b[:, 128 * cc : 128 * (cc + 1)],
                            identity=identb[:],
                        )
                        nc.scalar.copy(
                            out=hT[:, 128 * cc : 128 * (cc + 1)], in_=trp[:]
                        )
                    hrb = sp1.tile([128, 128], bf16, tag="hrb")
                    nc.sync.dma_start(out=hrb[:], in_=h_res_d[g, :, :])
                    trp = trp1.tile([128, 128], bf16, tag="tr", name=f"trhr_{g}")
                    nc.tensor.transpose(out=trp[:], in_=hrb[:], identity=identb[:])
                    nc.scalar.copy(out=hT[:, 512:640], in_=trp[:])
                    zps = zp1.tile([128, 128], f32, tag="z")
                    for cc in range(5):
                        nc.tensor.matmul(
                            zps[:],
                            lhsT=hT[:, 128 * cc : 128 * (cc + 1)],
                            rhs=wp1_s[cc][:],
                            start=(cc == 0),
                            stop=(cc == 4),
                        )
                    zsb = sp1.tile([128, 128], f32, tag="zsb")
                    nc.scalar.copy(out=zsb[:], in_=zps[:])
                    nc.vector.tensor_tensor(out=zsb[:], in0=zsb[:], in1=bp1r, op=OP.add)
                    nc.scalar.activation(zsb[:], zsb[:], AF.Gelu_apprx_tanh)
                    nc.vector.tensor_tensor(out=zsb[:], in0=zsb[:], in1=wp2r, op=OP.mult)
                    yred = sp1.tile([128, 1], f32, tag="yred")
                    nc.vector.tensor_reduce(yred[:], zsb[:], axis=AX.X, op=OP.add)
                    nc.vector.tensor_scalar(
                        out=ysb[:, g : g + 1],
                        in0=yred[:],
                        scalar1=bp2t[:, 0:1],
                        scalar2=None,
                        op0=OP.add,
                    )
            nc.sync.dma_start(out=y_d[:, :], in_=ysb[:])

    nc.compile()
    return nc


_CACHE = {}


def kernel(
    X,
    edge_index,
    edge_weight,
    W_in,
    b_in,
    lin0,
    att_src0,
    att_dst0,
    bias0,
    lin1,
    att_src1,
    att_dst1,
    bias1,
    Wp1,
    bp1,
    Wp2,
    bp2,
):
    import ml_dtypes
    from concourse.bass_utils import run_bass_kernel_spmd

    bfloat16 = ml_dtypes.bfloat16

    X = np.asarray(X, np.float32)
    perm, gidx, ewt, idx16, meta = _prep_graph(
        np.asarray(edge_index), np.asarray(edge_weight)
    )
    rhs0, rhs0l, wa1 = _prep_weights(
        np.asarray(W_in, np.float32),
        np.asarray(b_in, np.float32),
        np.asarray(lin0, np.float32),
        np.asarray(att_src0, np.float32),
        np.asarray(att_dst0, np.float32),
        np.asarray(lin1, np.float32),
        np.asarray(att_src1, np.float32),
        np.asarray(att_dst1, np.float32),
    )
    xptb = _build_xpt_blocks(X, perm)

    key = ("nc", meta["tot_cols"], tuple(meta["call_plan"]))
    if key not in _CACHE:
        _CACHE[key] = _build_nc(meta["call_plan"], meta["tot_cols"])
    nc = _CACHE[key]

    brow = np.zeros((1, 1280), np.float32)
    brow[0, 0:512] = np.asarray(bias0, np.float32)
    brow[0, 512:1024] = np.asarray(bias1, np.float32)
    brow[0, 1024:1152] = np.asarray(bp1, np.float32)
    brow[0, 1152:1280] = np.asarray(Wp2, np.float32)[:, 0]

    common = dict(
        xptb=xptb,
        rhs0=rhs0,
        rhs0l=rhs0l,
        lin0=np.ascontiguousarray(np.asarray(lin0, np.float32)).astype(bfloat16),
        lin1=np.ascontiguousarray(np.asarray(lin1, np.float32)).astype(bfloat16),
        wa1=wa1.astype(bfloat16),
        wp1=np.ascontiguousarray(np.asarray(Wp1, np.float32)).astype(bfloat16),
        brow=brow,
        bp2=np.asarray(bp2, np.float32).reshape(1, 1),
    )
    in_maps = [
        dict(common, xptl=xptb[G * r : G * (r + 1)], idx16=idx16[r], ewt=ewt[r])
        for r in range(R)
    ]

    res = run_bass_kernel_spmd(nc, in_maps, core_ids=list(range(R)), trace=False)

    y = np.zeros(N, np.float32)
    for r in range(R):
        yflat = res.results[r]["y"].T.reshape(-1)
        y[perm[r]] = yflat[DUM : DUM + perm[r].shape[0]]
    return y
